# revision 17
# baseline (speedup 1.0000x reference)
"""Trainium2 Bass kernel for a decoder LSTM (B=256, T=2048, HID=128, OUT=6).

Strategy: data-parallel over batch (8 cores x 32 batch) PLUS time-chunk
parallelism within each core. The LSTM forget-gate dynamics contract state
errors by ~10x per 4 steps, so the sequence is split into 16 chunks of
C=T/16 steps; chunks 1..15 start WU steps early from a zero state (warm-up)
and converge to the true trajectory well below the output tolerance.

The 16 chunks are organized as TWO phase-shifted chains of 8 chunks each.
Within a chain the 8 chunks advance together as extra batch columns: state
is [128 hidden partitions x 256 cols] (8 chunks x 32 batch) and the serial
recurrence is only C+WU steps. The two chains interleave on the engines
(one computes activations while the other runs its matmuls), so the wall
time approaches the ScalarE throughput bound instead of the chain latency.

Per step and chain: 4 gate matmuls accumulate onto a one-hot-prefilled
2-bank PSUM tile laid out [g|i|f|o], so sigmoid(i,f,o) is one ScalarE
instruction and tanh(g) a second; the cell update runs on VectorE; fc
logits accumulate 16 steps in PSUM before one bias-add copy; softmax is a
deferred phase.
"""

import os
import sys

for _p in ("/opt/trn_rl_repo", "/root/.axon_site/_ro/trn_rl_repo"):
    if os.path.isdir(_p) and _p not in sys.path:
        sys.path.insert(0, _p)

import numpy as np

B, T, VOCAB, EMB, HID, OUT = 256, 2048, 7, 20, 128, 6
NCORES = 8
BL = B // NCORES  # batch per core = 32
NCH = 2  # phase-shifted chains per core
KC = 8  # time chunks per chain
WD = KC * BL  # state width per chain = 256 cols
WU = 16  # warm-up steps for all chunks but the first
NFC = 16  # steps of fc logits accumulated per PSUM flush
GG, GI, GF, GO = 0, 1, 2, 3  # gate order in the PSUM tile: [g|i|f|o]
# PyTorch gate order in W_hh rows / table cols is (i,f,g,o)
PT_ORDER = {GI: 0, GF: 1, GG: 2, GO: 3}


def _split_overloaded_waits(nc, mybir, max_other=1):
    """walrus in this env rejects instructions with more than a couple of sem
    waits (and InstDrain with any). Move excess waits onto same-engine NoOps
    emitted just before; same-engine program order preserves semantics."""
    n_split = 0
    for f in nc.m.functions:
        for blk in f.blocks:
            out = []
            changed = False
            for inst in blk.instructions:
                si = inst.sync_info
                waits = list(si.on_wait) if si is not None and si.on_wait else []
                limit = 0 if isinstance(inst, mybir.InstDrain) else max_other
                if len(waits) > limit:
                    moved = waits if limit == 0 else waits[limit:]
                    keep = [] if limit == 0 else waits[:limit]
                    for i0, w in enumerate(moved):
                        nop = mybir.InstNoOp(
                            name=f"{inst.name}-wsplit{i0}", ins=[], outs=[]
                        )
                        nop.engine = inst.engine
                        nop.sync_info = mybir.SyncInfo(on_wait=[w], on_update=[])
                        out.append(nop)
                        n_split += 1
                    inst.sync_info = mybir.SyncInfo(
                        on_wait=keep,
                        on_update=list(si.on_update) if si.on_update else [],
                    )
                    changed = True
                out.append(inst)
            if changed:
                blk.instructions = out
    return n_split


def _patch_tile_drain():
    import concourse.tile as tile
    from concourse.vector_clock import ScopedClock, VectorClock

    def _drain_and_barrier_split(self, tick_clock, wait_clock):
        gc = tick_clock.global_clock
        n = len(gc)
        for j in range(n):
            if gc[j] <= 0:
                continue
            vec = [0] * n
            vec[j] = gc[j]
            nop = self.nc.sync.nop(nofuse=True, hint=f"drain_split_{j}")
            wait_clock.add_sem_waits(nop.ins, ScopedClock({None: VectorClock(vec)}))
        self.nc.sync.drain()
        self.nc.all_engine_barrier()
        assert self.sems is not None
        popped = self.nc._tile_sem_poison_stack.pop()
        assert popped is self._sem_poison
        self.nc.clear_and_free_semaphores(list(self.sems.allocated().values()))
        self.nc.all_engine_barrier()

    tile.TileContext._drain_and_barrier = _drain_and_barrier_split


_BUILD_CACHE = {}


def _n_iters(t_steps):
    assert t_steps % (NCH * KC) == 0
    c = t_steps // (NCH * KC)
    ni = c + WU
    # round iterations up so the fc flush granularity divides evenly
    ni = ((ni + NFC - 1) // NFC) * NFC
    return c, ni


def _build_nc(t_steps):
    if t_steps in _BUILD_CACHE:
        return _BUILD_CACHE[t_steps]
    import concourse.bass as bass
    import concourse.mybir as mybir
    import concourse.tile as tile

    _patch_tile_drain()

    f32 = mybir.dt.float32
    bf16 = mybir.dt.bfloat16
    AF = mybir.ActivationFunctionType
    C, NI = _n_iters(t_steps)

    nc = bass.Bass("TRN2", target_bir_lowering=False, debug=False)
    d_oh = nc.dram_tensor(
        "onehot", [VOCAB, NCH * NI * WD], bf16, kind="ExternalInput"
    )
    d_c0 = nc.dram_tensor("c0T", [HID, NCH * WD], bf16, kind="ExternalInput")
    d_w = nc.dram_tensor("w", [HID, 4 * HID], bf16, kind="ExternalInput")
    d_tbl = nc.dram_tensor("tbl", [VOCAB, 4 * HID], bf16, kind="ExternalInput")
    d_wfc = nc.dram_tensor("wfc", [HID, OUT], bf16, kind="ExternalInput")
    d_bfc = nc.dram_tensor("bfc", [128, OUT], f32, kind="ExternalInput")
    # out row p, chain ch, half h: chunk ch*8 + h*4 + p//32, batch p%32
    d_out = nc.dram_tensor(
        "out", [128, NCH, 2, NI, OUT], f32, kind="ExternalOutput"
    )

    with tile.TileContext(nc) as tc, tc.tile_pool(name="const", bufs=1) as constp:
        w_sb = constp.tile([HID, 4 * HID], bf16, name="w_sb")
        tbl_sb = constp.tile([VOCAB, 4 * HID], bf16, name="tbl_sb")
        wfc_sb = constp.tile([HID, OUT], bf16, name="wfc_sb")
        bfc_sb = constp.tile([128, OUT], f32, name="bfc_sb")
        oh_sb = constp.tile([VOCAB, NCH * NI * WD], bf16, name="oh_sb")
        h0_sb = constp.tile([HID, WD], bf16, name="h0_sb")
        scr = constp.tile([HID, WD], bf16, name="scr")
        cst = [constp.tile([HID, WD], bf16, name=f"cst{c_}") for c_ in range(NCH)]
        logit_sb = [
            constp.tile([128, 2 * NI * OUT], f32, name=f"logit{c_}")
            for c_ in range(NCH)
        ]
        probs_sb = [
            constp.tile([128, 2 * NI * OUT], f32, name=f"probs{c_}")
            for c_ in range(NCH)
        ]
        den_sb = [
            constp.tile([128, 2 * NI], f32, name=f"den{c_}") for c_ in range(NCH)
        ]

        nc.sync.dma_start(w_sb[:], d_w.ap())
        nc.sync.dma_start(tbl_sb[:], d_tbl.ap())
        nc.sync.dma_start(wfc_sb[:], d_wfc.ap())
        nc.sync.dma_start(bfc_sb[:], d_bfc.ap())
        for c_ in range(NCH):
            nc.sync.dma_start(
                cst[c_][:], d_c0.ap()[:, c_ * WD : (c_ + 1) * WD]
            )
        # one big load of the whole one-hot sequence (bf16, ~1MB over 7 rows)
        q = NCH * NI * WD // 4
        for kq in range(4):
            nc.sync.dma_start(
                oh_sb[:, kq * q : (kq + 1) * q], d_oh.ap()[:, kq * q : (kq + 1) * q]
            )
        nc.vector.memset(h0_sb[:], 0.0)
        # Pin the sigmoid_and_others table (contains tanh too) before the loop.
        nc.scalar.activation(scr[:], h0_sb[:], AF.Sigmoid)

        with (
            tc.tile_pool(name="ringp0", bufs=3) as ringp0,
            tc.tile_pool(name="ringp1", bufs=3) as ringp1,
            tc.tile_pool(name="gatep0", bufs=1, space="PSUM") as gatep0,
            tc.tile_pool(name="gatep1", bufs=1, space="PSUM") as gatep1,
            tc.tile_pool(name="fcp0", bufs=2, space="PSUM") as fcp0,
            tc.tile_pool(name="fcp1", bufs=2, space="PSUM") as fcp1,
            tc.tile_pool(name="workp0", bufs=2) as workp0,
            tc.tile_pool(name="workp1", bufs=2) as workp1,
        ):
            ringp = [ringp0, ringp1]
            gatep = [gatep0, gatep1]
            fcp = [fcp0, fcp1]
            workp = [workp0, workp1]
            h_prev = [h0_sb[:], h0_sb[:]]
            pfc = [None, None]
            pending_fc = [None, None]
            ps_cur = [None, None]

            def prefill(ch, j, ps):
                # input projection for step j: one-hot matmuls, one per gate.
                # PSUM accumulation groups are per-bank: start=True only on the
                # first matmul touching each bank (g,i share bank 0; f,o share
                # bank 1), stop=True only on the bank's last matmul.
                base = (ch * NI + j) * WD
                rhs = oh_sb[:, base : base + WD]
                for g in range(4):
                    pt = PT_ORDER[g]
                    nc.tensor.matmul(
                        ps[:, g * WD : (g + 1) * WD],
                        tbl_sb[:, pt * HID : (pt + 1) * HID],
                        rhs,
                        start=(g % 2 == 0),
                        stop=False,
                    )

            def emit_fc(ch, entry):
                jj, hs = entry
                jf = jj % NFC
                if jf == 0:
                    pfc[ch] = fcp[ch].tile(
                        [128, 2 * NFC * OUT], f32, tag="pfc", name=f"pfc{ch}"
                    )
                for half in range(2):
                    nc.tensor.matmul(
                        pfc[ch][
                            :, (half * NFC + jf) * OUT : (half * NFC + jf + 1) * OUT
                        ],
                        hs[:, half * 128 : (half + 1) * 128],
                        wfc_sb[:],
                        start=True,
                        stop=True,
                    )
                if jf == NFC - 1:
                    # flush: bias-add copy PSUM -> SBUF logits
                    for half in range(2):
                        dst = logit_sb[ch][
                            :,
                            (half * NI + (jj - jf)) * OUT : (half * NI + jj + 1)
                            * OUT,
                        ].rearrange("p (t o) -> p t o", o=OUT)
                        src = pfc[ch][
                            :, half * NFC * OUT : (half + 1) * NFC * OUT
                        ].rearrange("p (t o) -> p t o", o=OUT)
                        bias = bfc_sb[:].unsqueeze(1).broadcast_to([128, NFC, OUT])
                        nc.vector.scalar_tensor_tensor(
                            dst,
                            src,
                            1.0,
                            bias,
                            op0=mybir.AluOpType.mult,
                            op1=mybir.AluOpType.add,
                        )

            for ch in range(NCH):
                ps_cur[ch] = gatep[ch].tile(
                    [128, 4 * WD], f32, tag="ps", name=f"ps{ch}"
                )
                prefill(ch, 0, ps_cur[ch])

            for j in range(NI):
                for ch in range(NCH):
                    ps = ps_cur[ch]
                    # ---- gate matmuls (critical: need h_prev) ----
                    for g in range(4):
                        pt = PT_ORDER[g]
                        nc.tensor.matmul(
                            ps[:, g * WD : (g + 1) * WD],
                            w_sb[:, pt * HID : (pt + 1) * HID],
                            h_prev[ch],
                            start=False,
                            stop=(g % 2 == 1),
                        )
                    # fc for the previous step after this step's critical MMs
                    if pending_fc[ch] is not None:
                        emit_fc(ch, pending_fc[ch])
                        pending_fc[ch] = None
                    # ---- activations ----
                    sfio = workp[ch].tile([HID, 3 * WD], bf16, tag="sfio")
                    tg = workp[ch].tile([HID, WD], bf16, tag="tg")
                    ig = workp[ch].tile([HID, WD], bf16, tag="ig")
                    fcm = workp[ch].tile([HID, WD], bf16, tag="fcm")
                    tcl = workp[ch].tile([HID, WD], bf16, tag="tcl")
                    # tanh(g) first (it is ready first and feeds ig), then the
                    # sigmoid over the contiguous [i|f|o] region (one instr)
                    nc.scalar.activation(tg[:], ps[:, 0:WD], AF.Tanh)
                    nc.scalar.activation(sfio[:], ps[:, WD : 4 * WD], AF.Sigmoid)
                    si = sfio[:, 0:WD]
                    sf = sfio[:, WD : 2 * WD]
                    so = sfio[:, 2 * WD : 3 * WD]
                    # ---- cell update on VectorE ----
                    nc.vector.tensor_mul(fcm[:], sf, cst[ch][:])
                    nc.vector.tensor_mul(ig[:], si, tg[:])
                    nc.vector.tensor_add(cst[ch][:], fcm[:], ig[:])
                    nc.scalar.activation(tcl[:], cst[ch][:], AF.Tanh)
                    ring = ringp[ch].tile([HID, WD], bf16, tag="ring")
                    nc.vector.tensor_mul(ring[:], so, tcl[:])
                    h_prev[ch] = ring[:]
                    # ---- off-critical: prefill j+1 (same single psum tile,
                    # legal once this step's reads are done), defer fc ----
                    if j + 1 < NI:
                        prefill(ch, j + 1, ps)
                    pending_fc[ch] = (j, ring[:])
            for ch in range(NCH):
                if pending_fc[ch] is not None:
                    emit_fc(ch, pending_fc[ch])
                    pending_fc[ch] = None

        # ---- phase 2: softmax over OUT, one table switch to exp ----
        for ch in range(NCH):
            p3 = probs_sb[ch][:].rearrange("p (c o) -> p c o", o=OUT)
            nc.scalar.activation(probs_sb[ch][:], logit_sb[ch][:], AF.Exp)
            nc.vector.reduce_sum(den_sb[ch][:], p3, axis=mybir.AxisListType.X)
            nc.vector.reciprocal(den_sb[ch][:], den_sb[ch][:])
            rec_b = den_sb[ch][:].unsqueeze(2).broadcast_to([128, 2 * NI, OUT])
            nc.vector.tensor_mul(p3, p3, rec_b)
            p4 = probs_sb[ch][:].rearrange("p (h c o) -> p h c o", h=2, o=OUT)
            qn = NI // 4
            for kq in range(4):
                nc.sync.dma_start(
                    d_out.ap()[:, ch, :, kq * qn : (kq + 1) * qn, :],
                    p4[:, :, kq * qn : (kq + 1) * qn, :],
                )

    _split_overloaded_waits(nc, mybir)
    _BUILD_CACHE[t_steps] = nc
    return nc


def _host_prep(inputs, c0, W_ih, W_hh, b_ih, b_hh, W_fc, b_fc, emb, t_steps):
    import ml_dtypes

    bf16 = ml_dtypes.bfloat16
    inputs = np.asarray(inputs)
    C, NI = _n_iters(t_steps)
    table = (emb @ W_ih.T + (b_ih + b_hh)).astype(bf16)  # [7, 512]
    w = np.ascontiguousarray(W_hh.T.astype(bf16))  # [128, 512]
    wfc = np.ascontiguousarray(W_fc.T.astype(bf16))  # [128, 6]
    bfc = np.ascontiguousarray(np.tile(b_fc.astype(np.float32), (128, 1)))

    # global chunk g = ch*KC + k covers t in [g*C, (g+1)*C); chunk 0 is live
    # from j=0 (true init), all others warm up WU steps from zero state
    t_map = np.empty((NCH, KC, NI), np.int64)
    for ch in range(NCH):
        for k in range(KC):
            g = ch * KC + k
            if g == 0:
                t_map[ch, k] = np.minimum(np.arange(NI), t_steps - 1)
            else:
                t_map[ch, k] = np.clip(
                    g * C - WU + np.arange(NI), 0, t_steps - 1
                )

    in_maps = []
    for c in range(NCORES):
        idx = inputs[c * BL : (c + 1) * BL, :t_steps]  # [32, T]
        # chain ch, column (j, k, b) = ch*NI*WD + j*WD + k*BL + b
        oh = np.zeros((VOCAB, NCH * NI * WD), dtype=bf16)
        for ch in range(NCH):
            vals = idx[:, t_map[ch]]  # [32, KC, NI]
            vals = np.transpose(vals, (2, 1, 0)).reshape(-1)  # j, k, b
            cols = ch * NI * WD + np.arange(NI * WD)
            oh[vals, cols] = 1.0
        c0T = np.zeros((HID, NCH * WD), bf16)
        c0T[:, 0:BL] = c0[0, c * BL : (c + 1) * BL, :].T.astype(bf16)
        in_maps.append(
            {
                "onehot": oh,
                "c0T": np.ascontiguousarray(c0T),
                "w": w,
                "tbl": table,
                "wfc": wfc,
                "bfc": bfc,
            }
        )
    return in_maps


def _gather_output(res, t_steps):
    C, NI = _n_iters(t_steps)
    outs = []
    for c in range(NCORES):
        raw = res.results[c]["out"]  # [128, NCH, 2, NI, 6]
        core = np.empty((BL, t_steps, OUT), np.float32)
        for g in range(NCH * KC):
            ch, k = g // KC, g % KC
            rows = raw[(k % 4) * BL : (k % 4 + 1) * BL, ch, k // 4]  # [32, NI, 6]
            j0 = 0 if g == 0 else WU
            core[:, g * C : (g + 1) * C] = rows[:, j0 : j0 + C]
        outs.append(core)
    return np.concatenate(outs, axis=0)


def _run(inputs, c0, W_ih, W_hh, b_ih, b_hh, W_fc, b_fc, emb, t_steps=T,
         trace=False):
    from concourse.bass_utils import run_bass_kernel_spmd

    nc = _build_nc(t_steps)
    in_maps = _host_prep(
        inputs, c0, W_ih, W_hh, b_ih, b_hh, W_fc, b_fc, emb, t_steps
    )
    res = run_bass_kernel_spmd(
        nc, in_maps, core_ids=list(range(NCORES)), trace=trace
    )
    out = _gather_output(res, t_steps)
    return out, res


def kernel(inputs, c0, W_ih, W_hh, b_ih, b_hh, W_fc, b_fc, emb):
    out, _ = _run(
        np.asarray(inputs), np.asarray(c0), np.asarray(W_ih), np.asarray(W_hh),
        np.asarray(b_ih), np.asarray(b_hh), np.asarray(W_fc), np.asarray(b_fc),
        np.asarray(emb),
    )
    return out


# revision 31
# speedup vs baseline: 1.0144x; 1.0144x over previous
"""Trainium2 Bass kernel for a decoder LSTM (B=256, T=2048, HID=128, OUT=6).

Strategy: data-parallel over batch (8 cores x 32 batch) PLUS time-chunk
parallelism within each core. The LSTM forget-gate dynamics contract state
errors by ~10x per 4 steps, so the sequence is split into 16 chunks of
C=T/16 steps; chunks 1..15 start WU steps early from a zero state (warm-up)
and converge to the true trajectory well below the output tolerance.

The 16 chunks are organized as TWO phase-shifted chains of 8 chunks each.
Within a chain the 8 chunks advance together as extra batch columns: state
is [128 hidden partitions x 256 cols] (8 chunks x 32 batch) and the serial
recurrence is only C+WU steps. The two chains interleave on the engines
(one computes activations while the other runs its matmuls), so the wall
time approaches the ScalarE throughput bound instead of the chain latency.

Per step and chain: 4 gate matmuls accumulate onto a one-hot-prefilled
2-bank PSUM tile laid out [g|i|f|o], so sigmoid(i,f,o) is one ScalarE
instruction and tanh(g) a second; the cell update runs on VectorE; fc
logits accumulate 16 steps in PSUM before one bias-add copy; softmax is a
deferred phase.
"""

import os
import sys

for _p in ("/opt/trn_rl_repo", "/root/.axon_site/_ro/trn_rl_repo"):
    if os.path.isdir(_p) and _p not in sys.path:
        sys.path.insert(0, _p)

import numpy as np

B, T, VOCAB, EMB, HID, OUT = 256, 2048, 7, 20, 128, 6
NCORES = 8
BL = B // NCORES  # batch per core = 32
NCH = 2  # phase-shifted chains per core
KC = 8  # time chunks per chain
WD = KC * BL  # state width per chain = 256 cols
WU = 16  # warm-up steps for all chunks but the first
NFC = 16  # steps of fc logits accumulated per PSUM flush
GG, GI, GF, GO = 0, 1, 2, 3  # gate order in the PSUM tile: [g|i|f|o]
# PyTorch gate order in W_hh rows / table cols is (i,f,g,o)
PT_ORDER = {GI: 0, GF: 1, GG: 2, GO: 3}


def _split_overloaded_waits(nc, mybir, max_other=1):
    """walrus in this env rejects instructions with more than a couple of sem
    waits (and InstDrain with any). Move excess waits onto same-engine NoOps
    emitted just before; same-engine program order preserves semantics."""
    n_split = 0
    for f in nc.m.functions:
        for blk in f.blocks:
            out = []
            changed = False
            for inst in blk.instructions:
                si = inst.sync_info
                waits = list(si.on_wait) if si is not None and si.on_wait else []
                limit = 0 if isinstance(inst, mybir.InstDrain) else max_other
                if len(waits) > limit:
                    moved = waits if limit == 0 else waits[limit:]
                    keep = [] if limit == 0 else waits[:limit]
                    for i0, w in enumerate(moved):
                        nop = mybir.InstNoOp(
                            name=f"{inst.name}-wsplit{i0}", ins=[], outs=[]
                        )
                        nop.engine = inst.engine
                        nop.sync_info = mybir.SyncInfo(on_wait=[w], on_update=[])
                        out.append(nop)
                        n_split += 1
                    inst.sync_info = mybir.SyncInfo(
                        on_wait=keep,
                        on_update=list(si.on_update) if si.on_update else [],
                    )
                    changed = True
                out.append(inst)
            if changed:
                blk.instructions = out
    return n_split


def _patch_tile_drain():
    import concourse.tile as tile
    from concourse.vector_clock import ScopedClock, VectorClock

    def _drain_and_barrier_split(self, tick_clock, wait_clock):
        gc = tick_clock.global_clock
        n = len(gc)
        for j in range(n):
            if gc[j] <= 0:
                continue
            vec = [0] * n
            vec[j] = gc[j]
            nop = self.nc.sync.nop(nofuse=True, hint=f"drain_split_{j}")
            wait_clock.add_sem_waits(nop.ins, ScopedClock({None: VectorClock(vec)}))
        self.nc.sync.drain()
        self.nc.all_engine_barrier()
        assert self.sems is not None
        popped = self.nc._tile_sem_poison_stack.pop()
        assert popped is self._sem_poison
        self.nc.clear_and_free_semaphores(list(self.sems.allocated().values()))
        self.nc.all_engine_barrier()

    tile.TileContext._drain_and_barrier = _drain_and_barrier_split


_BUILD_CACHE = {}


def _n_iters(t_steps):
    assert t_steps % (NCH * KC) == 0
    c = t_steps // (NCH * KC)
    ni = c + WU
    # round iterations up so the fc flush granularity divides evenly
    ni = ((ni + NFC - 1) // NFC) * NFC
    return c, ni


def _build_nc(t_steps):
    if t_steps in _BUILD_CACHE:
        return _BUILD_CACHE[t_steps]
    import concourse.bass as bass
    import concourse.mybir as mybir
    import concourse.tile as tile

    _patch_tile_drain()

    f32 = mybir.dt.float32
    bf16 = mybir.dt.bfloat16
    AF = mybir.ActivationFunctionType
    C, NI = _n_iters(t_steps)

    nc = bass.Bass("TRN2", target_bir_lowering=False, debug=False)
    d_oh = nc.dram_tensor(
        "onehot", [VOCAB, NCH * NI * WD], bf16, kind="ExternalInput"
    )
    d_c0 = nc.dram_tensor("c0T", [HID, NCH * WD], bf16, kind="ExternalInput")
    d_w = nc.dram_tensor("w", [HID, 4 * HID], bf16, kind="ExternalInput")
    d_tbl = nc.dram_tensor("tbl", [VOCAB, 4 * HID], bf16, kind="ExternalInput")
    d_wfc = nc.dram_tensor("wfc", [HID, OUT], bf16, kind="ExternalInput")
    d_bfc = nc.dram_tensor("bfc", [128, OUT], f32, kind="ExternalInput")
    # out row p, chain ch, half h: chunk ch*8 + h*4 + p//32, batch p%32
    d_out = nc.dram_tensor(
        "out", [128, NCH, 2, NI, OUT], f32, kind="ExternalOutput"
    )

    with tile.TileContext(nc) as tc, tc.tile_pool(name="const", bufs=1) as constp:
        w_sb = constp.tile([HID, 4 * HID], bf16, name="w_sb")
        tbl_sb = constp.tile([VOCAB, 4 * HID], bf16, name="tbl_sb")
        wfc_sb = constp.tile([HID, OUT], bf16, name="wfc_sb")
        bfc_sb = constp.tile([128, OUT], f32, name="bfc_sb")
        oh_sb = constp.tile([VOCAB, NCH * NI * WD], bf16, name="oh_sb")
        h0_sb = constp.tile([HID, WD], bf16, name="h0_sb")
        scr = constp.tile([HID, WD], bf16, name="scr")
        cst = [constp.tile([HID, WD], bf16, name=f"cst{c_}") for c_ in range(NCH)]
        logit_sb = constp.tile([128, NCH * 2 * NI * OUT], f32, name="logit_sb")
        probs_sb = constp.tile([128, NCH * 2 * NI * OUT], f32, name="probs_sb")
        den_sb = constp.tile([128, NCH * 2 * NI], f32, name="den_sb")

        # the big one-hot load first, in 8 streamed chunks: prefill(u) only
        # depends on its own chunk, so compute starts once chunk 0 lands
        q = NCH * NI * WD // 8
        for kq in range(8):
            nc.sync.dma_start(
                oh_sb[:, kq * q : (kq + 1) * q], d_oh.ap()[:, kq * q : (kq + 1) * q]
            )
        nc.sync.dma_start(w_sb[:], d_w.ap())
        nc.sync.dma_start(tbl_sb[:], d_tbl.ap())
        nc.sync.dma_start(wfc_sb[:], d_wfc.ap())
        nc.sync.dma_start(bfc_sb[:], d_bfc.ap())
        for c_ in range(NCH):
            nc.sync.dma_start(
                cst[c_][:], d_c0.ap()[:, c_ * WD : (c_ + 1) * WD]
            )
        nc.vector.memset(h0_sb[:], 0.0)
        # Pin the sigmoid_and_others table (contains tanh too) before the loop.
        nc.scalar.activation(scr[:], h0_sb[:], AF.Sigmoid)

        with (
            # one SHARED ring pool: the buffer-reuse WAR dependency couples
            # the two chains so the scheduler cannot let one drift ahead
            tc.tile_pool(name="ringp", bufs=4) as ringp_shared,
            tc.tile_pool(name="gatep0", bufs=1, space="PSUM") as gatep0,
            tc.tile_pool(name="gatep1", bufs=1, space="PSUM") as gatep1,
            tc.tile_pool(name="fcp0", bufs=2, space="PSUM") as fcp0,
            tc.tile_pool(name="fcp1", bufs=2, space="PSUM") as fcp1,
            tc.tile_pool(name="workp0", bufs=2) as workp0,
            tc.tile_pool(name="workp1", bufs=2) as workp1,
        ):
            ringp = [ringp_shared, ringp_shared]
            gatep = [gatep0, gatep1]
            fcp = [fcp0, fcp1]
            workp = [workp0, workp1]
            h_prev = [h0_sb[:], h0_sb[:]]
            pfc = [None, None]
            pending_fc = [None, None]
            ps_cur = [None, None]

            def prefill(ch, j, ps):
                # input projection for step j: one-hot matmuls, one per gate.
                # PSUM accumulation groups are per-bank: start=True only on the
                # first matmul touching each bank (g,i share bank 0; f,o share
                # bank 1), stop=True only on the bank's last matmul.
                base = (ch * NI + j) * WD
                rhs = oh_sb[:, base : base + WD]
                for g in range(4):
                    pt = PT_ORDER[g]
                    nc.tensor.matmul(
                        ps[:, g * WD : (g + 1) * WD],
                        tbl_sb[:, pt * HID : (pt + 1) * HID],
                        rhs,
                        start=(g % 2 == 0),
                        stop=False,
                    )

            def emit_fc(ch, entry):
                jj, hs = entry
                jf = jj % NFC
                if jf == 0:
                    pfc[ch] = fcp[ch].tile(
                        [128, 2 * NFC * OUT], f32, tag="pfc", name=f"pfc{ch}"
                    )
                for half in range(2):
                    nc.tensor.matmul(
                        pfc[ch][
                            :, (half * NFC + jf) * OUT : (half * NFC + jf + 1) * OUT
                        ],
                        hs[:, half * 128 : (half + 1) * 128],
                        wfc_sb[:],
                        start=True,
                        stop=True,
                    )
                if jf == NFC - 1:
                    # flush: bias-add copy PSUM -> SBUF logits
                    for half in range(2):
                        lbase = (ch * 2 + half) * NI
                        dst = logit_sb[
                            :,
                            (lbase + (jj - jf)) * OUT : (lbase + jj + 1) * OUT,
                        ].rearrange("p (t o) -> p t o", o=OUT)
                        src = pfc[ch][
                            :, half * NFC * OUT : (half + 1) * NFC * OUT
                        ].rearrange("p (t o) -> p t o", o=OUT)
                        bias = bfc_sb[:].unsqueeze(1).broadcast_to([128, NFC, OUT])
                        nc.vector.scalar_tensor_tensor(
                            dst,
                            src,
                            1.0,
                            bias,
                            op0=mybir.AluOpType.mult,
                            op1=mybir.AluOpType.add,
                        )

            for ch in range(NCH):
                ps_cur[ch] = gatep[ch].tile(
                    [128, 4 * WD], f32, tag="ps", name=f"ps{ch}"
                )
                prefill(ch, 0, ps_cur[ch])

            for j in range(NI):
                for ch in range(NCH):
                    ps = ps_cur[ch]
                    # ---- gate matmuls (critical: need h_prev) ----
                    for g in range(4):
                        pt = PT_ORDER[g]
                        nc.tensor.matmul(
                            ps[:, g * WD : (g + 1) * WD],
                            w_sb[:, pt * HID : (pt + 1) * HID],
                            h_prev[ch],
                            start=False,
                            stop=(g % 2 == 1),
                        )
                    # fc for the previous step after this step's critical MMs
                    if pending_fc[ch] is not None:
                        emit_fc(ch, pending_fc[ch])
                        pending_fc[ch] = None
                    # ---- activations ----
                    sfio = workp[ch].tile([HID, 3 * WD], bf16, tag="sfio")
                    tg = workp[ch].tile([HID, WD], bf16, tag="tg")
                    ig = workp[ch].tile([HID, WD], bf16, tag="ig")
                    fcm = workp[ch].tile([HID, WD], bf16, tag="fcm")
                    tcl = workp[ch].tile([HID, WD], bf16, tag="tcl")
                    # tanh(g) first (it is ready first and feeds ig), then the
                    # sigmoid over the contiguous [i|f|o] region (one instr)
                    nc.scalar.activation(tg[:], ps[:, 0:WD], AF.Tanh)
                    nc.scalar.activation(sfio[:], ps[:, WD : 4 * WD], AF.Sigmoid)
                    si = sfio[:, 0:WD]
                    sf = sfio[:, WD : 2 * WD]
                    so = sfio[:, 2 * WD : 3 * WD]
                    # ---- cell update on VectorE ----
                    nc.vector.tensor_mul(fcm[:], sf, cst[ch][:])
                    nc.vector.tensor_mul(ig[:], si, tg[:])
                    nc.vector.tensor_add(cst[ch][:], fcm[:], ig[:])
                    nc.scalar.activation(tcl[:], cst[ch][:], AF.Tanh)
                    ring = ringp[ch].tile([HID, WD], bf16, tag="ring")
                    nc.vector.tensor_mul(ring[:], so, tcl[:])
                    h_prev[ch] = ring[:]
                    # ---- off-critical: prefill j+1 (same single psum tile,
                    # legal once this step's reads are done), defer fc ----
                    if j + 1 < NI:
                        prefill(ch, j + 1, ps)
                    pending_fc[ch] = (j, ring[:])
            for ch in range(NCH):
                if pending_fc[ch] is not None:
                    emit_fc(ch, pending_fc[ch])
                    pending_fc[ch] = None

        # ---- phase 2: softmax over OUT, one table switch to exp ----
        p3 = probs_sb[:].rearrange("p (c o) -> p c o", o=OUT)
        nc.scalar.activation(probs_sb[:], logit_sb[:], AF.Exp)
        nc.vector.reduce_sum(den_sb[:], p3, axis=mybir.AxisListType.X)
        nc.vector.reciprocal(den_sb[:], den_sb[:])
        rec_b = den_sb[:].unsqueeze(2).broadcast_to([128, NCH * 2 * NI, OUT])
        nc.vector.tensor_mul(p3, p3, rec_b)
        p4 = probs_sb[:].rearrange(
            "p (ch h c o) -> p ch h c o", ch=NCH, h=2, o=OUT
        )
        qn = NI // 4
        for kq in range(4):
            nc.sync.dma_start(
                d_out.ap()[:, :, :, kq * qn : (kq + 1) * qn, :],
                p4[:, :, :, kq * qn : (kq + 1) * qn, :],
            )

    _split_overloaded_waits(nc, mybir)
    _BUILD_CACHE[t_steps] = nc
    return nc


def _host_prep(inputs, c0, W_ih, W_hh, b_ih, b_hh, W_fc, b_fc, emb, t_steps):
    import ml_dtypes

    bf16 = ml_dtypes.bfloat16
    inputs = np.asarray(inputs)
    C, NI = _n_iters(t_steps)
    table = (emb @ W_ih.T + (b_ih + b_hh)).astype(bf16)  # [7, 512]
    w = np.ascontiguousarray(W_hh.T.astype(bf16))  # [128, 512]
    wfc = np.ascontiguousarray(W_fc.T.astype(bf16))  # [128, 6]
    bfc = np.ascontiguousarray(np.tile(b_fc.astype(np.float32), (128, 1)))

    # global chunk g = ch*KC + k covers t in [g*C, (g+1)*C); chunk 0 is live
    # from j=0 (true init), all others warm up WU steps from zero state
    t_map = np.empty((NCH, KC, NI), np.int64)
    for ch in range(NCH):
        for k in range(KC):
            g = ch * KC + k
            if g == 0:
                t_map[ch, k] = np.minimum(np.arange(NI), t_steps - 1)
            else:
                t_map[ch, k] = np.clip(
                    g * C - WU + np.arange(NI), 0, t_steps - 1
                )

    in_maps = []
    for c in range(NCORES):
        idx = inputs[c * BL : (c + 1) * BL, :t_steps]  # [32, T]
        # chain ch, column (j, k, b) = ch*NI*WD + j*WD + k*BL + b
        oh = np.zeros((VOCAB, NCH * NI * WD), dtype=bf16)
        for ch in range(NCH):
            vals = idx[:, t_map[ch]]  # [32, KC, NI]
            vals = np.transpose(vals, (2, 1, 0)).reshape(-1)  # j, k, b
            cols = ch * NI * WD + np.arange(NI * WD)
            oh[vals, cols] = 1.0
        c0T = np.zeros((HID, NCH * WD), bf16)
        c0T[:, 0:BL] = c0[0, c * BL : (c + 1) * BL, :].T.astype(bf16)
        in_maps.append(
            {
                "onehot": oh,
                "c0T": np.ascontiguousarray(c0T),
                "w": w,
                "tbl": table,
                "wfc": wfc,
                "bfc": bfc,
            }
        )
    return in_maps


def _gather_output(res, t_steps):
    C, NI = _n_iters(t_steps)
    outs = []
    for c in range(NCORES):
        raw = res.results[c]["out"]  # [128, NCH, 2, NI, 6]
        core = np.empty((BL, t_steps, OUT), np.float32)
        for g in range(NCH * KC):
            ch, k = g // KC, g % KC
            rows = raw[(k % 4) * BL : (k % 4 + 1) * BL, ch, k // 4]  # [32, NI, 6]
            j0 = 0 if g == 0 else WU
            core[:, g * C : (g + 1) * C] = rows[:, j0 : j0 + C]
        outs.append(core)
    return np.concatenate(outs, axis=0)


def _run(inputs, c0, W_ih, W_hh, b_ih, b_hh, W_fc, b_fc, emb, t_steps=T,
         trace=False):
    from concourse.bass_utils import run_bass_kernel_spmd

    nc = _build_nc(t_steps)
    in_maps = _host_prep(
        inputs, c0, W_ih, W_hh, b_ih, b_hh, W_fc, b_fc, emb, t_steps
    )
    res = run_bass_kernel_spmd(
        nc, in_maps, core_ids=list(range(NCORES)), trace=trace
    )
    out = _gather_output(res, t_steps)
    return out, res


def kernel(inputs, c0, W_ih, W_hh, b_ih, b_hh, W_fc, b_fc, emb):
    out, _ = _run(
        np.asarray(inputs), np.asarray(c0), np.asarray(W_ih), np.asarray(W_hh),
        np.asarray(b_ih), np.asarray(b_hh), np.asarray(W_fc), np.asarray(b_fc),
        np.asarray(emb),
    )
    return out
